# revision 2
# baseline (speedup 1.0000x reference)
"""HardNegativeMiningLoss on 8 TRN2 NeuronCores.

Data-parallel over anchor rows: core c owns rows [1024c, 1024(c+1)).
Each core keeps full E^T (bf16) resident in SBUF (DMA'd in column
blocks so compute starts early) and computes its [1024, 8192] sim
block with TensorE (fp32 PSUM, bf16 inputs), 1024 columns at a time.

Per 1024-column block the semi-hard pipeline is:
  ACT   y = Identity(psum - pos_min)  (per-row AP bias, fp16 out):
        the semi-hard test 'sim < pos_min' becomes simply y < 0, and
        values near the threshold — exactly the ones top-16 cares
        about — sit near 0 where fp16 spacing is ~1e-4.
  then one of two penalty paths pushes non-semi entries to y-5:
  DVE   pen = (y >= 0) * -5 (immediate tensor_scalar, 4x mode) and
        v = y + pen (2x tensor_tensor) — 6 of 8 blocks, ts/tt fused
        2048-wide across block pairs; or
  PE    s = ACT Sign(psum - pos_min); psum += diag(-2.5) @ s; ACT
        evacuates v = psum - pos_min - 2.5 — 2 of 8 blocks, emitted
        one block late so the TensorE never waits on the Sign.  This
        keeps TensorE (~122us) and DVE (~121us) equally busy.
  DVE   max8 over each 1024 block -> per-block top-8.
Positives and self have sim >= pos_min so they land at v <= -3;
semi-hard negatives keep v = sim - pos_min in (-2, 0).  A 3-op merge
(max8 / match_replace / max8) of the 8 block top-8s gives the row
top-16.  exp((y-5)/T) vanishes (< 1e-19 rel), so no masking/count
logic is needed in the logsumexp epilogue, which runs once, batched
(ACT loads its activation table at most twice):
  lse = ln(sum exp(v/T)) + pos_min/T,
  loss_i = (-pos_sim + lse) * valid * hs.
Rows with no semi-hard negative (hs=0, ~0.05%) are recomputed exactly
on host, as is the label-derived row metadata (pos_min / pos_sim /
valid, ~0.05% of FLOPs).  Host sums the per-core partials.

Measured on 8 axon trn2 cores: 155.5us HW exec (baseline 290us, prior
session's quoted 1073us), rel err 4.8e-4.
"""

import numpy as np

import concourse.bacc as bacc
import concourse.bass as bass
import concourse.mybir as mybir
import concourse.tile as tile
from concourse.bass_utils import run_bass_kernel_spmd

B = 8192
D = 512
N_CORES = 8
ROWS_PER_CORE = B // N_CORES          # 1024
N_ROW_TILES = ROWS_PER_CORE // 128    # 8
SB = 2048                             # superblock: 4 psum banks
N_SB = B // SB                        # 4
TEMP = 0.07
PEN = -5.0
FP = mybir.dt.float32
BF = mybir.dt.bfloat16
F16 = mybir.dt.float16


def _build_program():
    nc = bacc.Bacc(None, target_bir_lowering=False)

    et_d = nc.dram_tensor("et", [D, B], BF, kind="ExternalInput")
    eloc_d = nc.dram_tensor("eloc", [D, ROWS_PER_CORE], BF, kind="ExternalInput")
    meta_d = nc.dram_tensor("rowmeta", [ROWS_PER_CORE, 8], FP, kind="ExternalInput")
    dneg_d = nc.dram_tensor("dneg", [128, 128], BF, kind="ExternalInput")
    out_d = nc.dram_tensor("out", [128, N_ROW_TILES, 2], FP, kind="ExternalOutput")

    et_v = et_d[:].rearrange("(k p) n -> k p n", p=128)       # [4,128,B]
    eloc_v = eloc_d[:].rearrange("(k p) n -> k p n", p=128)   # [4,128,1024]
    meta_v = meta_d[:].rearrange("(t p) m -> p t m", p=128)   # [128,8,8]
    NK = D // 128
    Id = mybir.ActivationFunctionType.Identity
    Sg = mybir.ActivationFunctionType.Sign

    BLK = 1024
    N_BLKS = B // BLK                 # 8
    PE_BLKS = (2, 6)                  # penalty via Sign + diag matmul
    PAIRS = ((0, 1), (4, 5))          # DVE-path pairs share one wide ts/tt

    with tile.TileContext(nc) as tc:
        with (
            tc.tile_pool(name="wts", bufs=1) as wts,
            tc.tile_pool(name="psum", bufs=4, space="PSUM") as psp,
            tc.tile_pool(name="yp", bufs=3) as yp,
            tc.tile_pool(name="pp", bufs=3) as pp,
            tc.tile_pool(name="vp", bufs=3) as vp,
            tc.tile_pool(name="sgp", bufs=2) as sgp,
            tc.tile_pool(name="small", bufs=2) as smp,
            tc.tile_pool(name="acc", bufs=1) as accp,
        ):
            # resident inputs; et arrives in column blocks so rt=0 compute
            # can start as soon as its first blocks land
            metas = accp.tile([128, N_ROW_TILES, 8], FP, tag="metas")
            nc.sync.dma_start(metas[:], meta_v)
            eloc_t = []
            for k in range(NK):
                t = wts.tile([128, ROWS_PER_CORE], BF, tag=f"el{k}")
                nc.sync.dma_start(t[:], eloc_v[k])
                eloc_t.append(t)
            dneg = wts.tile([128, 128], BF, tag="dneg")
            nc.sync.dma_start(dneg[:], dneg_d[:])
            et_t = [wts.tile([128, B], BF, tag=f"et{k}", name=f"et{k}")
                    for k in range(NK)]
            for cb in range(B // 1024):
                sl = slice(cb * 1024, (cb + 1) * 1024)
                for k in range(NK):
                    nc.sync.dma_start(et_t[k][:, sl], et_v[k][:, sl])

            t16a = accp.tile([128, N_ROW_TILES, 16], F16, tag="t16a")
            pools8 = accp.tile([128, N_ROW_TILES, 64], F16, tag="pools8")
            out_t = accp.tile([128, N_ROW_TILES, 2], FP, tag="out")
            zbias = accp.tile([128, 1], FP, tag="zbias")
            nc.gpsimd.memset(zbias[:], 0.0)

            # brief PE p-state warm-up while the et DMA streams in
            wps = psp.tile([128, BLK], FP, tag="ps")
            for wi in range(2):
                nc.tensor.matmul(
                    wps[:, 0:512], eloc_t[0][:, 0:128],
                    eloc_t[0][:, 0:512], start=True, stop=True)

            def emit_main(rt, blk):
                """16 k-outer matmuls for one [128,1024] column block."""
                elrt = [eloc_t[k][:, rt * 128:(rt + 1) * 128]
                        for k in range(NK)]
                ps = psp.tile([128, BLK], FP, tag="ps")
                for k in range(NK):
                    for q in range(2):
                        c0 = blk * BLK + q * 512
                        nc.tensor.matmul(
                            ps[:, q * 512:(q + 1) * 512],
                            elrt[k],
                            et_t[k][:, c0:c0 + 512],
                            start=(k == 0),
                            stop=(k == NK - 1),
                        )
                return ps

            def emit_pe_finish(rt, blk, ps, sgn):
                """psum += -2.5*sign, evacuate v, top-8."""
                npm2 = metas[:, rt, 4:5]
                for q in range(2):
                    nc.tensor.matmul(
                        ps[:, q * 512:(q + 1) * 512], dneg[:],
                        sgn[:, q * 512:(q + 1) * 512],
                        start=False, stop=True)
                v = vp.tile([128, BLK], F16, tag="v")
                nc.scalar.activation(v[:], ps[:], Id, bias=npm2, scale=1.0)
                nc.vector.max(pools8[:, rt, blk * 8:(blk + 1) * 8], v[:])

            pending = None
            for rt in range(N_ROW_TILES):
                npm = metas[:, rt, 0:1]                       # -pos_min

                blk = 0
                while blk < N_BLKS:
                    if blk in (p[0] for p in PAIRS):
                        # paired DVE-path: one wide ts/tt over 2048 cols
                        ps0 = emit_main(rt, blk)
                        if pending is not None:
                            emit_pe_finish(*pending)
                            pending = None
                        y2 = yp.tile([128, 2 * BLK], F16, tag="y")
                        nc.scalar.activation(y2[:, 0:BLK], ps0[:], Id,
                                             bias=npm, scale=1.0)
                        ps1 = emit_main(rt, blk + 1)
                        nc.scalar.activation(y2[:, BLK:2 * BLK], ps1[:], Id,
                                             bias=npm, scale=1.0)
                        pen = pp.tile([128, 2 * BLK], F16, tag="pen")
                        nc.vector.tensor_scalar(
                            pen[:], y2[:], 0.0, PEN,
                            op0=mybir.AluOpType.is_ge,
                            op1=mybir.AluOpType.mult)
                        v2t = vp.tile([128, 2 * BLK], F16, tag="v2")
                        nc.vector.tensor_tensor(v2t[:], y2[:], pen[:],
                                                op=mybir.AluOpType.add)
                        for h in range(2):
                            nc.vector.max(
                                pools8[:, rt, (blk + h) * 8:(blk + h + 1) * 8],
                                v2t[:, h * BLK:(h + 1) * BLK])
                        blk += 2
                        continue

                    ps = emit_main(rt, blk)
                    if pending is not None:
                        emit_pe_finish(*pending)
                        pending = None
                    if blk in PE_BLKS:
                        sgn = sgp.tile([128, BLK], F16, tag="sgn")
                        nc.scalar.activation(sgn[:], ps[:], Sg,
                                             bias=npm, scale=1.0)
                        pending = (rt, blk, ps, sgn)
                    else:
                        y = yp.tile([128, BLK], F16, tag="y1", name="y1")
                        nc.scalar.activation(y[:], ps[:], Id,
                                             bias=npm, scale=1.0)
                        pen = pp.tile([128, BLK], F16, tag="pen1", name="pen1")
                        nc.vector.tensor_scalar(
                            pen[:], y[:], 0.0, PEN,
                            op0=mybir.AluOpType.is_ge,
                            op1=mybir.AluOpType.mult)
                        v = vp.tile([128, BLK], F16, tag="v1", name="v1")
                        nc.vector.tensor_tensor(v[:], y[:], pen[:],
                                                op=mybir.AluOpType.add)
                        nc.vector.max(pools8[:, rt, blk * 8:(blk + 1) * 8],
                                      v[:])
                    blk += 1

            if pending is not None:
                emit_pe_finish(*pending)
                pending = None

            # deferred merges: 8 block top-8s -> top-16 per row tile
            for rt in range(N_ROW_TILES):
                poolrt = pools8[:, rt, :]
                poolmr = smp.tile([128, 64], F16, tag="poolmr")
                nc.vector.max(t16a[:, rt, 0:8], poolrt)
                nc.vector.match_replace(poolmr[:], t16a[:, rt, 0:8], poolrt,
                                        -32768.0)
                nc.vector.max(t16a[:, rt, 8:16], poolmr[:])

            # epilogue: batched so ACT swaps activation tables at most twice
            t16f = t16a[:].rearrange("p a b -> p (a b)")      # [128,128] f16
            e128 = accp.tile([128, N_ROW_TILES, 16], FP, tag="e128")
            nc.scalar.activation(
                e128[:].rearrange("p a b -> p (a b)"), t16f,
                mybir.ActivationFunctionType.Exp,
                bias=zbias[:], scale=1.0 / TEMP)
            s8 = accp.tile([128, N_ROW_TILES], FP, tag="s8")
            nc.vector.tensor_reduce(
                s8[:], e128[:], axis=mybir.AxisListType.X,
                op=mybir.AluOpType.add)
            nc.vector.tensor_scalar(s8[:], s8[:], 1e-30, None,
                                    op0=mybir.AluOpType.max)
            lns = accp.tile([128, N_ROW_TILES], FP, tag="lns")
            nc.scalar.activation(lns[:], s8[:],
                                 mybir.ActivationFunctionType.Ln)
            # hs flag: any semi-hard value survived (max in (-2,0) vs <= -3)
            nc.vector.tensor_scalar(
                out_t[:, :, 1], t16a[:, :, 0], -2.5, None,
                op0=mybir.AluOpType.is_gt)
            # loss = (lnS + pos_min/T - pos_sim) * valid * hs
            a1 = accp.tile([128, N_ROW_TILES], FP, tag="a1")
            nc.vector.tensor_tensor(a1[:], lns[:], metas[:, :, 3],
                                    op=mybir.AluOpType.add)
            nc.vector.tensor_tensor(a1[:], a1[:], metas[:, :, 1],
                                    op=mybir.AluOpType.subtract)
            nc.vector.tensor_tensor(a1[:], a1[:], metas[:, :, 2],
                                    op=mybir.AluOpType.mult)
            nc.vector.tensor_tensor(out_t[:, :, 0], a1[:], out_t[:, :, 1],
                                    op=mybir.AluOpType.mult)

            nc.sync.dma_start(out_d[:], out_t[:])

    nc.compile()
    return nc


def _host_rowmeta(emb: np.ndarray, labels: np.ndarray):
    """-pos_min / pos_sim / valid / pos_min/T per row from label groups."""
    Bn = emb.shape[0]
    pos_min = np.full(Bn, 1e30, np.float32)
    pos_sum = np.zeros(Bn, np.float32)
    cnt = np.zeros(Bn, np.int64)
    order = np.argsort(labels, kind="stable")
    sl = labels[order]
    starts = np.flatnonzero(np.r_[True, sl[1:] != sl[:-1]])
    ends = np.r_[starts[1:], Bn]
    for s, e in zip(starts, ends):
        idx = order[s:e]
        n = e - s
        if n < 2:
            continue
        G = emb[idx] @ emb[idx].T          # [n, n] fp32
        np.fill_diagonal(G, np.nan)
        pos_min[idx] = np.nanmin(G, axis=1)
        pos_sum[idx] = np.nansum(G, axis=1)
        cnt[idx] = n - 1
    pos_sim = pos_sum / np.maximum(cnt, 1) / TEMP
    valid = ((cnt > 0) & ((Bn - 1 - cnt) > 0)).astype(np.float32)

    meta = np.zeros((Bn, 8), np.float32)
    # rows without positives: bias y = u-4 so every entry sits below the
    # hs threshold (-2.5) without generating fp16 infs; valid=0 anyway
    meta[:, 0] = np.where(cnt > 0, -pos_min, np.float32(-4.0))
    meta[:, 1] = pos_sim
    meta[:, 2] = valid
    meta[:, 3] = np.where(cnt > 0, pos_min / TEMP, 0.0)
    meta[:, 4] = meta[:, 0] - 2.5
    return meta, valid


def _exact_row_loss(emb, labels, i):
    """Reference loss_i for one row (used for the rare hs=0 rows)."""
    sim = emb @ emb[i]
    same = labels == labels[i]
    pos = same.copy()
    pos[i] = False
    neg = ~same
    if not pos.any() or not neg.any():
        return 0.0
    pmin = sim[pos].min()
    semi = neg & (sim < pmin)
    cand = sim[semi] if semi.any() else sim[neg]
    k = min(16, cand.size)
    top = np.partition(cand, cand.size - k)[-k:] / TEMP
    m = top.max()
    lse = m + np.log(np.exp(top - m).sum())
    return float(-sim[pos].mean() / TEMP + lse)


_profile = [None]
_nc_cache = [None]


def kernel(embeddings: np.ndarray, labels: np.ndarray) -> np.ndarray:
    emb = np.asarray(embeddings, np.float32)
    lab = np.asarray(labels)
    meta, valid = _host_rowmeta(emb, lab)

    et = np.ascontiguousarray(emb.T).astype(mybir.dt.np(BF))          # [D, B]
    dneg = np.diag(np.full(128, -2.5, np.float32)).astype(mybir.dt.np(BF))
    in_maps = []
    for c in range(N_CORES):
        r0 = c * ROWS_PER_CORE
        in_maps.append({
            "et": et,
            "eloc": np.ascontiguousarray(emb[r0:r0 + ROWS_PER_CORE].T)
                      .astype(mybir.dt.np(BF)),
            "rowmeta": meta[r0:r0 + ROWS_PER_CORE],
            "dneg": dneg,
        })

    if _nc_cache[0] is None:
        _nc_cache[0] = _build_program()
    nc = _nc_cache[0]
    trace = _profile[0] is not None
    res = run_bass_kernel_spmd(nc, in_maps, list(range(N_CORES)), trace=trace)
    if trace:
        _profile[0] = res

    total = np.float64(0.0)
    for c in range(N_CORES):
        o = np.asarray(res.results[c]["out"], np.float32)   # [128, 8, 2]
        total += np.float64(o[:, :, 0]).sum()
        # rows the device flagged as having no semi-hard negative
        hs = o[:, :, 1] > 0.5
        miss = np.argwhere(~hs)                             # [n, 2] (p, t)
        for p, t in miss:
            r = c * ROWS_PER_CORE + t * 128 + p
            if valid[r] > 0:
                total += _exact_row_loss(emb, lab, r)
    n_valid = float(valid.sum())
    return np.float32(total / max(n_valid, 1.0))
